# revision 1
# baseline (speedup 1.0000x reference)
"""Trainium2 Bass kernel: multi-resolution 3D feature-grid trilinear lookup.

reference: for 4 cubic grids g_l (16, r, r, r), r in {16,32,64,128},
trilinearly interpolate at 2M points x in [-1,1]^3 (align_corners=True),
concat features -> (2M, 64) f32.

Architecture (8 NeuronCores, data-parallel over points):
- Host builds, per level, an 8x corner-duplicated table: row[cell] =
  [c=16][k=8 corners] f32 = 512B, so ONE dma_gather element fetches all
  8 trilinear corners of a cell.
- dma_gather (GPSIMD "mlp"-library ucode, 4 SWDGE queues = the throughput
  limit) uses int16 indices, so tables are addressed in 32768-row segments.
  The host deals points round-robin over cores within z-buckets
  (seg2(z), seg3(z)); each bucket gets a statically sized point window per
  core, so ONE SPMD program has a fully static window -> segment map.
  Windows are padded with copies of a valid point (idx stays in range,
  outputs discarded on host).
- Host precomputes all gather indices (int16, [16, n/16] wrap, replicated
  to the 8 GPSIMD partition groups) and the permuted x stream; kernel
  output is inverse-permuted on the host.
- Device per window: lerp weights on VectorE (magic-number floor),
  per level one dma_gather + weight-prescale mul + fused 8-corner reduce,
  then one output DMA.
"""
import math
import os
import sys

import numpy as np

for _p in ("/opt/trn_rl_repo",):
    if _p not in sys.path and os.path.isdir(_p):
        sys.path.insert(0, _p)

P = 128
C = 16
RS = [16, 32, 64, 128]
L = len(RS)
OC = C * L
N_POINTS = 2_000_000
N_CORES = 8
SEGROWS = 32768
WMAX = 6144              # max points per window (dma_gather num_idxs cap)
MAGIC = np.float32(12582912.0)   # 1.5 * 2**23
CFLOOR = np.float32(0.49999997)

_NC_CACHE = {}
_RUNNER_CACHE = {}
_LAST_INFO = {}


# ---------------------------------------------------------------- host math
def _i0_f32(u, s):
    """Device-exact floor of u*s (u = x+1 as f32): rint(fl(u*s) - CFLOOR)."""
    fxm = u * np.float32(s)
    return np.rint(fxm - CFLOOR).astype(np.int64)


def _rows_of_points(x):
    """Per level: int64 table row ids, shape (L, N)."""
    u = x.astype(np.float32) + np.float32(1.0)
    rows = []
    for r in RS:
        s = np.float32(0.5 * (r - 1))
        ix = _i0_f32(u[:, 0], s)
        iy = _i0_f32(u[:, 1], s)
        iz = _i0_f32(u[:, 2], s)
        rows.append((iz * r + iy) * r + ix)
    return np.stack(rows)


def _buckets_of_z(z):
    u = z.astype(np.float32) + np.float32(1.0)
    z3 = _i0_f32(u, 0.5 * (RS[3] - 1))
    z2 = _i0_f32(u, 0.5 * (RS[2] - 1))
    return (z2 >> 3) * 64 + (z3 >> 1)


def _enumerate_runs():
    """Static list of (seg2, seg3, z_repr, prob) over z in [-1, 1)."""
    zs = np.linspace(-1.0, 1.0, 1 << 20, endpoint=False)
    zf = zs.astype(np.float32)
    b = _buckets_of_z(zf)
    change = np.nonzero(np.diff(b))[0]
    starts = np.concatenate([[0], change + 1])
    ends = np.concatenate([change + 1, [len(zs)]])
    runs = []
    for a, e in zip(starts, ends):
        mid = (a + e) // 2
        runs.append((int(b[mid] >> 6), int(b[mid] & 63), float(zf[mid]),
                     (e - a) / len(zs)))
    return runs


RUNS = _enumerate_runs()
RUN_BUCKET = np.array([(s2 * 64 + s3) for s2, s3, _, _ in RUNS])


def _capacities(counts=None):
    """Per-core point capacity per run (multiple of 128)."""
    caps = []
    for i, (s2, s3, zr, p) in enumerate(RUNS):
        mu = (N_POINTS * p / N_CORES) if counts is None else counts[i]
        cap = mu + 5.0 * math.sqrt(max(mu, 4.0)) + 32
        caps.append(int(math.ceil(cap / P) * P))
    return tuple(caps)


CAPS0 = _capacities()


def _window_table(caps):
    """Static windows: list of (size, seg2, seg3, pt_off, x_off, idx_off,
    out_off, idxcols)."""
    wins = []
    pt = 0
    for cap, (s2, s3, _, _) in zip(caps, RUNS):
        off = 0
        while off < cap:
            sz = min(WMAX, cap - off)
            wins.append([sz, s2, s3, pt])
            pt += sz
            off += sz
    # byte/element offsets
    x_off = 0
    idx_off = 0
    out_off = 0
    full = []
    for sz, s2, s3, pt0 in wins:
        ncol = sz // 16
        full.append((sz, s2, s3, pt0, x_off, idx_off, out_off, ncol))
        x_off += P * 3 * (sz // P)
        idx_off += P * L * ncol
        out_off += P * OC * (sz // P)
    tot = pt
    return full, tot, x_off, idx_off, out_off


# ------------------------------------------------------------- device build
def _build_nc(caps):
    from concourse import bass, bacc, mybir, tile
    from concourse import library_config as lc

    f32 = mybir.dt.float32
    i16 = mybir.dt.int16
    AP = bass.AP
    mult = mybir.AluOpType.mult
    add = mybir.AluOpType.add
    sub = mybir.AluOpType.subtract

    wins, tot, x_elems, idx_elems, out_elems = _window_table(caps)

    nc = bacc.Bacc("TRN2", num_swdge_queues=4)
    x_d = nc.dram_tensor("x", [x_elems], f32, kind="ExternalInput")
    idx_d = nc.dram_tensor("idx", [idx_elems], i16, kind="ExternalInput")
    tabs = [
        nc.dram_tensor(f"t{l}", [RS[l] ** 3 * 8 * C], f32,
                       kind="ExternalInput")
        for l in range(L)
    ]
    sr_d = nc.dram_tensor("sr", [P, L], f32, kind="ExternalInput")
    out_d = nc.dram_tensor("out", [out_elems], f32, kind="ExternalOutput")

    def v(base_ap, off, dims):
        return AP(base_ap.tensor, base_ap.offset + off,
                  [list(d) for d in dims])

    with tile.TileContext(nc) as tc:
        with (
            tc.tile_pool(name="const", bufs=1) as cpool,
            tc.tile_pool(name="work", bufs=2) as wpool,
            tc.tile_pool(name="gat", bufs=4) as gpool,
            tc.tile_pool(name="osb", bufs=2) as opool,
        ):
            nc.gpsimd.load_library(lc.mlp)
            sr = cpool.tile([P, L], f32)
            nc.sync.dma_start(out=sr[:], in_=sr_d[:])

            for win_i, (sz, s2, s3, pt0, xo, io, oo, ncol) in enumerate(wins):
                J = sz // P
                xt = wpool.tile([P, 3 * J], f32, tag="xt")
                nc.sync.dma_start(
                    out=xt[:],
                    in_=v(x_d[:], xo, [(3 * J, P), (1, 3 * J)]))
                ix = wpool.tile([P, L * ncol], i16, tag="ix")
                nc.sync.dma_start(
                    out=ix[:],
                    in_=v(idx_d[:], io, [(L * ncol, P), (1, L * ncol)]))

                # u = x + 1
                u = wpool.tile([P, 3 * J], f32, tag="u")
                nc.scalar.activation(
                    out=u[:], in_=xt[:],
                    func=mybir.ActivationFunctionType.Copy, bias=1.0)
                # fxm[l, (t,c)] = u * s_l
                fxm = wpool.tile([P, L, 3 * J], f32, tag="fxm")
                nc.vector.tensor_tensor(
                    out=fxm[:],
                    in0=v(u[:], 0, [(3 * J, P), (0, L), (1, 3 * J)]),
                    in1=v(sr[:], 0, [(L, P), (1, L), (0, 3 * J)]),
                    op=mult)
                # magic floor: i0f = rint(fxm - CFLOOR)
                t2 = wpool.tile([P, L, 3 * J], f32, tag="t2")
                nc.vector.tensor_scalar(
                    out=t2[:], in0=fxm[:],
                    scalar1=float(-CFLOOR), scalar2=float(MAGIC),
                    op0=add, op1=add)
                i0f = wpool.tile([P, L, 3, J], f32, tag="i0f")
                fxm_iter = v(fxm[:], 0,
                             [(3 * L * J, P), (3 * J, L), (3, J), (1, 3)])
                t2_iter = v(t2[:], 0,
                            [(3 * L * J, P), (3 * J, L), (3, J), (1, 3)])
                i0f_out = v(i0f[:], 0,
                            [(3 * L * J, P), (3 * J, L), (1, J), (J, 3)])
                nc.vector.tensor_scalar(
                    out=i0f_out, in0=t2_iter,
                    scalar1=float(-MAGIC), scalar2=None, op0=add)
                # wp[l, axis, pair, t]: pair1 = w = fxm - i0f, pair0 = 1 - w
                wp = wpool.tile([P, L, 3, 2, J], f32, tag="wp")
                w_out = v(wp[:], J,
                          [(6 * L * J, P), (6 * J, L), (1, J), (2 * J, 3)])
                i0f_iter = v(i0f[:], 0,
                             [(3 * L * J, P), (3 * J, L), (1, J), (J, 3)])
                nc.vector.tensor_tensor(
                    out=w_out, in0=fxm_iter, in1=i0f_iter, op=sub)
                w_nat = v(wp[:], J,
                          [(6 * L * J, P), (6 * J, L), (2 * J, 3), (1, J)])
                om_nat = v(wp[:], 0,
                           [(6 * L * J, P), (6 * J, L), (2 * J, 3), (1, J)])
                nc.vector.tensor_scalar(
                    out=om_nat, in0=w_nat, scalar1=-1.0, scalar2=1.0,
                    op0=mult, op1=add)

                osb = opool.tile([P, J, OC], f32, tag="osb")

                for l in range(L):
                    # wzy[k4, t] = zpair x ypair (k4 = dz*2 + dy)
                    wzy = wpool.tile([P, 4, J], f32, tag="wzy")
                    zoff = l * 6 * J + 2 * 2 * J
                    yoff = l * 6 * J + 1 * 2 * J
                    xoff = l * 6 * J + 0 * 2 * J
                    nc.vector.tensor_tensor(
                        out=v(wzy[:], 0,
                              [(4 * J, P), (2 * J, 2), (J, 2), (1, J)]),
                        in0=v(wp[:], zoff,
                              [(6 * L * J, P), (J, 2), (0, 2), (1, J)]),
                        in1=v(wp[:], yoff,
                              [(6 * L * J, P), (0, 2), (J, 2), (1, J)]),
                        op=mult)
                    # w8[t, k8] = wzy[k4, t] * xpair(dx), k8 = 4dz+2dy+dx
                    w8 = wpool.tile([P, J, 8], f32, tag="w8")
                    nc.vector.tensor_tensor(
                        out=v(w8[:], 0,
                              [(8 * J, P), (8, J), (2, 4), (1, 2)]),
                        in0=v(wzy[:], 0,
                              [(4 * J, P), (1, J), (J, 4), (0, 2)]),
                        in1=v(wp[:], xoff,
                              [(6 * L * J, P), (1, J), (0, 4), (J, 2)]),
                        op=mult)

                    # gather (512B per point = 8 corners x 16 feats)
                    seg = s3 if l == 3 else (s2 if l == 2 else 0)
                    base = seg * SEGROWS
                    rows = min(SEGROWS, RS[l] ** 3 - base)
                    g = gpool.tile([P, J * 8 * C], f32, tag="g")
                    nc.gpsimd.dma_gather(
                        out_ap=v(g[:], 0,
                                 [(J * 8 * C, P), (8 * C, J), (1, 8 * C)]),
                        in_ap=v(tabs[l][:], base * 8 * C,
                                [(8 * C, rows), (1, 8 * C)]),
                        idxs_ap=ix[:, l * ncol:(l + 1) * ncol],
                        num_idxs=sz, num_idxs_reg=sz,
                        elem_size=8 * C, single_packet=False,
                        queue_num=l)

                    # in-place prescale: g[t, c, k] *= w8[t, k]
                    nc.vector.tensor_tensor(
                        out=v(g[:], 0,
                              [(128 * J, P), (128, J), (8, C), (1, 8)]),
                        in0=v(g[:], 0,
                              [(128 * J, P), (128, J), (8, C), (1, 8)]),
                        in1=v(w8[:], 0,
                              [(8 * J, P), (8, J), (0, C), (1, 8)]),
                        op=mult)
                    # fused reduce over 8 corners -> osb[:, :, l*16:+16]
                    nc.vector.tensor_reduce(
                        out=v(osb[:], l * C, [(J * OC, P), (OC, J), (1, C)]),
                        in_=v(g[:], 0, [(128 * J, P), (8, C * J), (1, 8)]),
                        axis=mybir.AxisListType.X,
                        op=add)

                nc.sync.dma_start(
                    out=v(out_d[:], oo, [(J * OC, P), (1, J * OC)]),
                    in_=osb[:])

    nc.finalize()
    return nc


def _get_nc(caps):
    if caps not in _NC_CACHE:
        _NC_CACHE[caps] = _build_nc(caps)
    return _NC_CACHE[caps]


# ---------------------------------------------------------------- host prep
def _make_tables(grids):
    tabs = []
    for G in grids:
        C_, D, H, W = G.shape
        Gp = np.pad(G, ((0, 0), (0, 1), (0, 1), (0, 1)), mode="edge")
        T = np.empty((D, H, W, C_, 8), dtype=np.float32)
        for k in range(8):
            dz, dy, dx = (k >> 2) & 1, (k >> 1) & 1, k & 1
            T[..., k] = np.moveaxis(
                Gp[:, dz:dz + D, dy:dy + H, dx:dx + W], 0, -1)
        tabs.append(np.ascontiguousarray(
            T.reshape(D * H * W * 8 * C_).astype(np.float32)))
    return tabs


def _prep_core_inputs(x, caps, wins, tot, x_elems, idx_elems):
    """Deal points to cores round-robin within each run; build per-core
    x / idx arrays and the inverse map."""
    bucket = _buckets_of_z(np.ascontiguousarray(x[:, 2]))
    rows = _rows_of_points(x)          # (L, N) int64
    order = np.argsort(bucket, kind="stable")
    bsort = bucket[order]
    run_of_bucket = {int(b): i for i, b in enumerate(RUN_BUCKET)}

    # per-run contiguous slices of `order`
    run_starts = np.searchsorted(bsort, RUN_BUCKET, side="left")
    run_ends = np.searchsorted(bsort, RUN_BUCKET, side="right")
    counts = run_ends - run_starts

    # overflow?
    for i, (a, e) in enumerate(zip(run_starts, run_ends)):
        need = math.ceil((e - a) / N_CORES)
        if need > caps[i]:
            return None, counts  # caller rebuilds with larger caps
    if int(counts.sum()) != len(x):
        # points in buckets outside the enumerated runs (shouldn't happen)
        return None, None

    xs = np.zeros((N_CORES, x_elems), np.float32)
    idxs = np.zeros((N_CORES, idx_elems), np.int16)
    srcmap = np.full((N_CORES, tot), -1, np.int64)

    # window lookup per run: list of (win) for each run in order
    run_wins = {}
    wi = 0
    for ri, cap in enumerate(caps):
        lst = []
        off = 0
        while off < cap:
            lst.append(wins[wi])
            off += wins[wi][0]
            wi += 1
        run_wins[ri] = lst

    for ri in range(len(RUNS)):
        a, e = int(run_starts[ri]), int(run_ends[ri])
        pts = order[a:e]
        for c in range(N_CORES):
            mine = pts[c::N_CORES]
            n = len(mine)
            capn = caps[ri]
            if n == 0:
                # synth point in this run
                zr = RUNS[ri][2]
                fill_ids = None
            # build padded id list of length capn
            ids = np.empty(capn, np.int64)
            if n > 0:
                ids[:n] = mine
                ids[n:] = mine[-1] if n else 0
            else:
                ids[:] = -2  # synth marker
            # scatter into this run's windows
            woff = 0
            for (szw, s2, s3, pt0, xo, io, oo, ncol) in run_wins[ri]:
                sel = ids[woff:woff + szw]
                Jw = szw // P
                # x block, p-major: slot (p, t) = point t*128+p
                if n > 0:
                    xblk = x[sel].astype(np.float32)
                    rl = rows[:, sel]
                else:
                    zr = np.float32(RUNS[ri][2])
                    xblk = np.zeros((szw, 3), np.float32)
                    xblk[:, 2] = zr
                    rl = _rows_of_points(xblk)
                # local rows per level
                loc = np.empty((L, szw), np.int64)
                for l in range(L):
                    seg = s3 if l == 3 else (s2 if l == 2 else 0)
                    loc[l] = rl[l] - seg * SEGROWS
                assert loc.min() >= 0 and loc.max() < SEGROWS, (
                    ri, loc.min(), loc.max())
                # x: [szw pts] -> [P, Jw, 3] p-major
                xs[c, xo:xo + szw * 3] = (
                    xblk.reshape(Jw, P, 3).transpose(1, 0, 2).reshape(-1))
                # idx: per level wrap [16, ncolw] replicated x8,
                # stored partition-major [P, L, ncolw]
                ncolw = szw // 16
                iblk = np.empty((P, L, ncolw), np.int16)
                for l in range(L):
                    wrapped = loc[l].astype(np.int16).reshape(ncolw, 16).T
                    iblk[:, l, :] = np.tile(wrapped, (8, 1))
                idxs[c, io:io + P * L * ncolw] = iblk.reshape(-1)
                if n > 0:
                    vmask = np.arange(woff, woff + szw) < n
                    srcmap[c, pt0:pt0 + szw][vmask] = sel[:max(
                        0, min(n - woff, szw))]
                woff += szw
    return (xs, idxs, srcmap), counts


# -------------------------------------------------------------------- runner
def _make_runner(nc):
    import jax
    from jax.sharding import Mesh, PartitionSpec, NamedSharding
    from jax.experimental.shard_map import shard_map
    from concourse import bass2jax, mybir

    bass2jax.install_neuronx_cc_hook()
    partition_name = (nc.partition_id_tensor.name
                      if nc.partition_id_tensor else None)
    in_names, out_names, out_avals, out_shapes = [], [], [], []
    for alloc in nc.m.functions[0].allocations:
        if not isinstance(alloc, mybir.MemoryLocationSet):
            continue
        name = alloc.memorylocations[0].name
        if alloc.kind == "ExternalInput":
            if name != partition_name:
                in_names.append(name)
        elif alloc.kind == "ExternalOutput":
            shape = tuple(alloc.tensor_shape)
            dtype = mybir.dt.np(alloc.dtype)
            out_names.append(name)
            out_avals.append(jax.core.ShapedArray(shape, dtype))
            out_shapes.append((shape, dtype))
    n_params = len(in_names)
    all_in = list(in_names) + list(out_names)
    if partition_name is not None:
        all_in.append(partition_name)

    def _body(*args):
        operands = list(args)
        if partition_name is not None:
            operands.append(bass2jax.partition_id_tensor())
        outs = bass2jax._bass_exec_p.bind(
            *operands,
            out_avals=tuple(out_avals),
            in_names=tuple(all_in),
            out_names=tuple(out_names),
            lowering_input_output_aliases=(),
            sim_require_finite=True,
            sim_require_nnan=True,
            nc=nc,
        )
        return tuple(outs)

    devices = jax.devices()[:N_CORES]
    mesh = Mesh(np.asarray(devices), ("core",))
    spec = PartitionSpec("core")
    n_outs = len(out_names)
    sharded = jax.jit(
        shard_map(_body, mesh=mesh,
                  in_specs=(spec,) * (n_params + n_outs),
                  out_specs=(spec,) * n_outs, check_rep=False),
        keep_unused=True)
    shard = NamedSharding(mesh, spec)
    return sharded, shard, in_names, out_names, out_shapes


def _get_runner(caps):
    if caps not in _RUNNER_CACHE:
        _RUNNER_CACHE[caps] = _make_runner(_get_nc(caps))
    return _RUNNER_CACHE[caps]


# -------------------------------------------------------------------- kernel
def kernel(**inputs):
    import jax
    global _LAST_INFO

    x = np.ascontiguousarray(np.asarray(inputs["x"], dtype=np.float32))
    grids = [np.asarray(inputs[f"g{i}"], dtype=np.float32) for i in range(L)]
    assert x.shape == (N_POINTS, 3)

    caps = CAPS0
    wins, tot, x_elems, idx_elems, out_elems = _window_table(caps)
    prep, counts = _prep_core_inputs(x, caps, wins, tot, x_elems, idx_elems)
    if prep is None:
        # data-driven capacities (rare fallback; recompiles)
        caps = _capacities([math.ceil(c / N_CORES) * 1.05 + 64
                            for c in counts])
        wins, tot, x_elems, idx_elems, out_elems = _window_table(caps)
        prep, counts = _prep_core_inputs(x, caps, wins, tot, x_elems,
                                         idx_elems)
        assert prep is not None
    xs, idxs, srcmap = prep

    tabs = _make_tables(grids)
    sr = np.zeros((P, L), np.float32)
    for l, r in enumerate(RS):
        sr[:, l] = 0.5 * (r - 1)

    sharded, shard, in_names, out_names, out_shapes = _get_runner(caps)
    devices = jax.devices()[:N_CORES]

    def put_sharded(percore_list):
        g = tuple(
            (N_CORES * percore_list[0].shape[0],) + percore_list[0].shape[1:])
        parts = [jax.device_put(a, d) for a, d in zip(percore_list, devices)]
        return jax.make_array_from_single_device_arrays(g, shard, parts)

    per_core = {"x": xs, "idx": idxs}
    args = []
    for name in in_names:
        if name in per_core:
            args.append(put_sharded(
                [np.ascontiguousarray(per_core[name][c])
                 for c in range(N_CORES)]))
        elif name == "sr":
            args.append(put_sharded([sr] * N_CORES))
        elif name.startswith("t"):
            args.append(put_sharded([tabs[int(name[1])]] * N_CORES))
        else:
            raise KeyError(name)
    for shape, dtype in out_shapes:
        z = np.zeros(shape, dtype)
        args.append(put_sharded([z] * N_CORES))
    jax.block_until_ready(args)

    _LAST_INFO = {"caps": caps, "args": args, "sharded": sharded,
                  "out_shapes": out_shapes}

    outs = sharded(*args)
    jax.block_until_ready(outs)

    out_flat = np.asarray(outs[0]).reshape(N_CORES, out_elems)
    result = np.empty((N_POINTS, OC), np.float32)
    # per core: walk windows, de-permute p-major [P, Jw, OC] -> stream order
    for ci in range(N_CORES):
        oc = out_flat[ci]
        sm = srcmap[ci]
        for (szw, s2, s3, pt0, xo, io, oo, ncol) in wins:
            Jw = szw // P
            blk = oc[oo:oo + szw * OC].reshape(P, Jw, OC)
            ids = sm[pt0:pt0 + szw]
            valid = ids >= 0
            if valid.any():
                stream = blk.transpose(1, 0, 2).reshape(szw, OC)
                result[ids[valid]] = stream[valid]
    return np.ascontiguousarray(result)



# revision 8
# speedup vs baseline: 14.2776x; 14.2776x over previous
"""Trainium2 Bass kernel: multi-resolution 3D feature-grid trilinear lookup.

reference: for 4 cubic grids g_l (16, r, r, r), r in {16,32,64,128},
trilinearly interpolate at 2M points x in [-1,1]^3 (align_corners=True),
concat features -> (2M, 64) f32.

Architecture (8 NeuronCores, data-parallel over points):
- Host builds, per level, an 8x corner-duplicated table: row[cell] =
  [c=16][k=8 corners] f32 = 512B, so ONE dma_gather element fetches all
  8 trilinear corners of a cell.
- dma_gather (GPSIMD "mlp"-library ucode, 4 SWDGE queues = the throughput
  limit) uses int16 indices, so tables are addressed in 32768-row segments.
  The host deals points round-robin over cores within z-buckets
  (seg2(z), seg3(z)); each bucket gets a statically sized point window per
  core, so ONE SPMD program has a fully static window -> segment map.
  Windows are padded with copies of a valid point (idx stays in range,
  outputs discarded on host).
- Host precomputes all gather indices (int16, [16, n/16] wrap, replicated
  to the 8 GPSIMD partition groups) and the permuted x stream; kernel
  output is inverse-permuted on the host.
- Device per window: lerp weights on VectorE (magic-number floor),
  per level one dma_gather + weight-prescale mul + fused 8-corner reduce,
  then one output DMA.
"""
import math
import os
import sys

import numpy as np

for _p in ("/opt/trn_rl_repo",):
    if _p not in sys.path and os.path.isdir(_p):
        sys.path.insert(0, _p)

P = 128
C = 16
RS = [16, 32, 64, 128]
L = len(RS)
OC = C * L
N_POINTS = 2_000_000
N_CORES = 8
SEGROWS = 32768
WMAX = 6144              # max points per window (dma_gather num_idxs cap)
MAGIC = np.float32(12582912.0)   # 1.5 * 2**23
CFLOOR = np.float32(0.49999997)

_NC_CACHE = {}
_RUNNER_CACHE = {}
_LAST_INFO = {}

# Sort points within each (run, core) slice by finest-level table row id so
# gather addresses are near-sequential (HBM row-buffer locality). Host-only;
# the srcmap de-permutation already handles arbitrary order.
SORT_WINDOWS = True


# ---------------------------------------------------------------- host math
def _i0_f32(u, s):
    """Device-exact floor of u*s (u = x+1 as f32): rint(fl(u*s) - CFLOOR)."""
    fxm = u * np.float32(s)
    return np.rint(fxm - CFLOOR).astype(np.int64)


def _rows_of_points(x):
    """Per level: int64 table row ids, shape (L, N)."""
    u = x.astype(np.float32) + np.float32(1.0)
    rows = []
    for r in RS:
        s = np.float32(0.5 * (r - 1))
        ix = _i0_f32(u[:, 0], s)
        iy = _i0_f32(u[:, 1], s)
        iz = _i0_f32(u[:, 2], s)
        rows.append((iz * r + iy) * r + ix)
    return np.stack(rows)


def _buckets_of_z(z):
    u = z.astype(np.float32) + np.float32(1.0)
    z3 = _i0_f32(u, 0.5 * (RS[3] - 1))
    z2 = _i0_f32(u, 0.5 * (RS[2] - 1))
    return (z2 >> 3) * 64 + (z3 >> 1)


def _enumerate_runs():
    """Static list of (seg2, seg3, z_repr, prob) over z in [-1, 1)."""
    zs = np.linspace(-1.0, 1.0, 1 << 20, endpoint=False)
    zf = zs.astype(np.float32)
    b = _buckets_of_z(zf)
    change = np.nonzero(np.diff(b))[0]
    starts = np.concatenate([[0], change + 1])
    ends = np.concatenate([change + 1, [len(zs)]])
    runs = []
    for a, e in zip(starts, ends):
        mid = (a + e) // 2
        runs.append((int(b[mid] >> 6), int(b[mid] & 63), float(zf[mid]),
                     (e - a) / len(zs)))
    return runs


RUNS = _enumerate_runs()
RUN_BUCKET = np.array([(s2 * 64 + s3) for s2, s3, _, _ in RUNS])


def _capacities(counts=None):
    """Per-core point capacity per run (multiple of 128)."""
    caps = []
    for i, (s2, s3, zr, p) in enumerate(RUNS):
        mu = (N_POINTS * p / N_CORES) if counts is None else counts[i]
        cap = mu + 5.0 * math.sqrt(max(mu, 4.0)) + 32
        caps.append(int(math.ceil(cap / P) * P))
    return tuple(caps)


CAPS0 = _capacities()


def _window_table(caps):
    """Static windows: list of (size, seg2, seg3, pt_off, x_off, idx_off,
    out_off, idxcols)."""
    wins = []
    pt = 0
    for cap, (s2, s3, _, _) in zip(caps, RUNS):
        off = 0
        while off < cap:
            sz = min(WMAX, cap - off)
            wins.append([sz, s2, s3, pt])
            pt += sz
            off += sz
    # byte/element offsets
    x_off = 0
    idx_off = 0
    out_off = 0
    full = []
    for sz, s2, s3, pt0 in wins:
        ncol = sz // 16
        full.append((sz, s2, s3, pt0, x_off, idx_off, out_off, ncol))
        x_off += P * 3 * (sz // P)
        idx_off += P * L * ncol
        out_off += P * OC * (sz // P)
    tot = pt
    return full, tot, x_off, idx_off, out_off


# ------------------------------------------------------------- device build
def _build_nc(caps):
    from concourse import bass, bacc, mybir, tile
    from concourse import library_config as lc

    f32 = mybir.dt.float32
    bf16 = mybir.dt.bfloat16
    i16 = mybir.dt.int16
    AP = bass.AP
    mult = mybir.AluOpType.mult
    add = mybir.AluOpType.add
    sub = mybir.AluOpType.subtract

    wins, tot, x_elems, idx_elems, out_elems = _window_table(caps)

    nc = bacc.Bacc("TRN2", num_swdge_queues=4)
    x_d = nc.dram_tensor("x", [x_elems], f32, kind="ExternalInput")
    idx_d = nc.dram_tensor("idx", [idx_elems], i16, kind="ExternalInput")
    # bf16 tables: HBM random reads have ~512B min granularity so 256B rows
    # cost the same DMA time as 512B, but halve SBUF footprint (deeper
    # gather pipelining) and double DVE throughput on prescale/reduce.
    tabs = [
        nc.dram_tensor(f"t{l}", [RS[l] ** 3 * 8 * C], bf16,
                       kind="ExternalInput")
        for l in range(L)
    ]
    sr_d = nc.dram_tensor("sr", [P, L], f32, kind="ExternalInput")
    out_d = nc.dram_tensor("out", [out_elems], f32, kind="ExternalOutput")

    def v(base_ap, off, dims):
        return AP(base_ap.tensor, base_ap.offset + off,
                  [list(d) for d in dims])

    with tile.TileContext(nc) as tc:
        with (
            tc.tile_pool(name="const", bufs=1) as cpool,
            tc.tile_pool(name="work", bufs=3) as wpool,
            tc.tile_pool(name="gat", bufs=12) as gpool,
            tc.tile_pool(name="osb", bufs=3) as opool,
        ):
            nc.gpsimd.load_library(lc.mlp)
            sr = cpool.tile([P, L], f32)
            nc.sync.dma_start(out=sr[:], in_=sr_d[:])

            for win_i, (sz, s2, s3, pt0, xo, io, oo, ncol) in enumerate(wins):
                J = sz // P
                xt = wpool.tile([P, 3 * J], f32, tag="xt")
                nc.sync.dma_start(
                    out=xt[:],
                    in_=v(x_d[:], xo, [(3 * J, P), (1, 3 * J)]))
                ix = wpool.tile([P, L * ncol], i16, tag="ix")
                nc.sync.dma_start(
                    out=ix[:],
                    in_=v(idx_d[:], io, [(L * ncol, P), (1, L * ncol)]))

                # u = x + 1
                u = wpool.tile([P, 3 * J], f32, tag="u")
                nc.scalar.activation(
                    out=u[:], in_=xt[:],
                    func=mybir.ActivationFunctionType.Copy, bias=1.0)
                # fxm[l, (t,c)] = u * s_l
                fxm = wpool.tile([P, L, 3 * J], f32, tag="fxm")
                nc.vector.tensor_tensor(
                    out=fxm[:],
                    in0=v(u[:], 0, [(3 * J, P), (0, L), (1, 3 * J)]),
                    in1=v(sr[:], 0, [(L, P), (1, L), (0, 3 * J)]),
                    op=mult)
                # magic floor: i0f = rint(fxm - CFLOOR)
                t2 = wpool.tile([P, L, 3 * J], f32, tag="t2")
                nc.vector.tensor_scalar(
                    out=t2[:], in0=fxm[:],
                    scalar1=float(-CFLOOR), scalar2=float(MAGIC),
                    op0=add, op1=add)
                i0f = wpool.tile([P, L, 3, J], f32, tag="i0f")
                fxm_iter = v(fxm[:], 0,
                             [(3 * L * J, P), (3 * J, L), (3, J), (1, 3)])
                t2_iter = v(t2[:], 0,
                            [(3 * L * J, P), (3 * J, L), (3, J), (1, 3)])
                i0f_out = v(i0f[:], 0,
                            [(3 * L * J, P), (3 * J, L), (1, J), (J, 3)])
                nc.vector.tensor_scalar(
                    out=i0f_out, in0=t2_iter,
                    scalar1=float(-MAGIC), scalar2=None, op0=add)
                # wp[l, axis, pair, t]: pair1 = w = fxm - i0f, pair0 = 1 - w
                wp = wpool.tile([P, L, 3, 2, J], f32, tag="wp")
                w_out = v(wp[:], J,
                          [(6 * L * J, P), (6 * J, L), (1, J), (2 * J, 3)])
                i0f_iter = v(i0f[:], 0,
                             [(3 * L * J, P), (3 * J, L), (1, J), (J, 3)])
                nc.vector.tensor_tensor(
                    out=w_out, in0=fxm_iter, in1=i0f_iter, op=sub)
                w_nat = v(wp[:], J,
                          [(6 * L * J, P), (6 * J, L), (2 * J, 3), (1, J)])
                om_nat = v(wp[:], 0,
                           [(6 * L * J, P), (6 * J, L), (2 * J, 3), (1, J)])
                nc.vector.tensor_scalar(
                    out=om_nat, in0=w_nat, scalar1=-1.0, scalar2=1.0,
                    op0=mult, op1=add)

                osb = opool.tile([P, J, OC], f32, tag="osb")

                for l in range(L):
                    # wzy[k4, t] = zpair x ypair (k4 = dz*2 + dy)
                    wzy = wpool.tile([P, 4, J], f32, tag="wzy")
                    zoff = l * 6 * J + 2 * 2 * J
                    yoff = l * 6 * J + 1 * 2 * J
                    xoff = l * 6 * J + 0 * 2 * J
                    nc.vector.tensor_tensor(
                        out=v(wzy[:], 0,
                              [(4 * J, P), (2 * J, 2), (J, 2), (1, J)]),
                        in0=v(wp[:], zoff,
                              [(6 * L * J, P), (J, 2), (0, 2), (1, J)]),
                        in1=v(wp[:], yoff,
                              [(6 * L * J, P), (0, 2), (J, 2), (1, J)]),
                        op=mult)
                    # w8[t, k8] = wzy[k4, t] * xpair(dx), k8 = 4dz+2dy+dx
                    w8 = wpool.tile([P, J, 8], bf16, tag="w8")
                    nc.vector.tensor_tensor(
                        out=v(w8[:], 0,
                              [(8 * J, P), (8, J), (2, 4), (1, 2)]),
                        in0=v(wzy[:], 0,
                              [(4 * J, P), (1, J), (J, 4), (0, 2)]),
                        in1=v(wp[:], xoff,
                              [(6 * L * J, P), (1, J), (0, 4), (J, 2)]),
                        op=mult)

                    # gather (256B per point = 8 corners x 16 bf16 feats)
                    seg = s3 if l == 3 else (s2 if l == 2 else 0)
                    base = seg * SEGROWS
                    rows = min(SEGROWS, RS[l] ** 3 - base)
                    g = gpool.tile([P, J * 8 * C], bf16, tag="g")
                    nc.gpsimd.dma_gather(
                        out_ap=v(g[:], 0,
                                 [(J * 8 * C, P), (8 * C, J), (1, 8 * C)]),
                        in_ap=v(tabs[l][:], base * 8 * C,
                                [(8 * C, rows), (1, 8 * C)]),
                        idxs_ap=ix[:, l * ncol:(l + 1) * ncol],
                        num_idxs=sz, num_idxs_reg=sz,
                        elem_size=8 * C, single_packet=False,
                        queue_num=l)

                    # in-place prescale: g[t, c, k] *= w8[t, k]
                    nc.vector.tensor_tensor(
                        out=v(g[:], 0,
                              [(128 * J, P), (128, J), (8, C), (1, 8)]),
                        in0=v(g[:], 0,
                              [(128 * J, P), (128, J), (8, C), (1, 8)]),
                        in1=v(w8[:], 0,
                              [(8 * J, P), (8, J), (0, C), (1, 8)]),
                        op=mult)
                    # fused reduce over 8 corners -> osb[:, :, l*16:+16]
                    nc.vector.tensor_reduce(
                        out=v(osb[:], l * C, [(J * OC, P), (OC, J), (1, C)]),
                        in_=v(g[:], 0, [(128 * J, P), (8, C * J), (1, 8)]),
                        axis=mybir.AxisListType.X,
                        op=add)

                nc.sync.dma_start(
                    out=v(out_d[:], oo, [(J * OC, P), (1, J * OC)]),
                    in_=osb[:])

    nc.finalize()
    return nc


def _get_nc(caps):
    if caps not in _NC_CACHE:
        _NC_CACHE[caps] = _build_nc(caps)
    return _NC_CACHE[caps]


# ---------------------------------------------------------------- host prep
def _make_tables(grids):
    import ml_dtypes
    tabs = []
    for G in grids:
        C_, D, H, W = G.shape
        Gp = np.pad(G, ((0, 0), (0, 1), (0, 1), (0, 1)), mode="edge")
        T = np.empty((D, H, W, C_, 8), dtype=np.float32)
        for k in range(8):
            dz, dy, dx = (k >> 2) & 1, (k >> 1) & 1, k & 1
            T[..., k] = np.moveaxis(
                Gp[:, dz:dz + D, dy:dy + H, dx:dx + W], 0, -1)
        tabs.append(np.ascontiguousarray(
            T.reshape(D * H * W * 8 * C_).astype(ml_dtypes.bfloat16)))
    return tabs


def _prep_core_inputs(x, caps, wins, tot, x_elems, idx_elems):
    """Deal points to cores round-robin within each run; build per-core
    x / idx arrays and the inverse map."""
    bucket = _buckets_of_z(np.ascontiguousarray(x[:, 2]))
    rows = _rows_of_points(x)          # (L, N) int64
    order = np.argsort(bucket, kind="stable")
    bsort = bucket[order]
    run_of_bucket = {int(b): i for i, b in enumerate(RUN_BUCKET)}

    # per-run contiguous slices of `order`
    run_starts = np.searchsorted(bsort, RUN_BUCKET, side="left")
    run_ends = np.searchsorted(bsort, RUN_BUCKET, side="right")
    counts = run_ends - run_starts

    # overflow?
    for i, (a, e) in enumerate(zip(run_starts, run_ends)):
        need = math.ceil((e - a) / N_CORES)
        if need > caps[i]:
            return None, counts  # caller rebuilds with larger caps
    if int(counts.sum()) != len(x):
        # points in buckets outside the enumerated runs (shouldn't happen)
        return None, None

    xs = np.zeros((N_CORES, x_elems), np.float32)
    idxs = np.zeros((N_CORES, idx_elems), np.int16)
    srcmap = np.full((N_CORES, tot), -1, np.int64)

    # window lookup per run: list of (win) for each run in order
    run_wins = {}
    wi = 0
    for ri, cap in enumerate(caps):
        lst = []
        off = 0
        while off < cap:
            lst.append(wins[wi])
            off += wins[wi][0]
            wi += 1
        run_wins[ri] = lst

    for ri in range(len(RUNS)):
        a, e = int(run_starts[ri]), int(run_ends[ri])
        pts = order[a:e]
        for c in range(N_CORES):
            mine = pts[c::N_CORES]
            if SORT_WINDOWS and len(mine) > 1:
                mine = mine[np.argsort(rows[3, mine], kind="stable")]
            n = len(mine)
            capn = caps[ri]
            if n == 0:
                # synth point in this run
                zr = RUNS[ri][2]
                fill_ids = None
            # build padded id list of length capn
            ids = np.empty(capn, np.int64)
            if n > 0:
                ids[:n] = mine
                ids[n:] = mine[-1] if n else 0
            else:
                ids[:] = -2  # synth marker
            # scatter into this run's windows
            woff = 0
            for (szw, s2, s3, pt0, xo, io, oo, ncol) in run_wins[ri]:
                sel = ids[woff:woff + szw]
                Jw = szw // P
                # x block, p-major: slot (p, t) = point t*128+p
                if n > 0:
                    xblk = x[sel].astype(np.float32)
                    rl = rows[:, sel]
                else:
                    zr = np.float32(RUNS[ri][2])
                    xblk = np.zeros((szw, 3), np.float32)
                    xblk[:, 2] = zr
                    rl = _rows_of_points(xblk)
                # local rows per level
                loc = np.empty((L, szw), np.int64)
                for l in range(L):
                    seg = s3 if l == 3 else (s2 if l == 2 else 0)
                    loc[l] = rl[l] - seg * SEGROWS
                assert loc.min() >= 0 and loc.max() < SEGROWS, (
                    ri, loc.min(), loc.max())
                # x: [szw pts] -> [P, Jw, 3] p-major
                xs[c, xo:xo + szw * 3] = (
                    xblk.reshape(Jw, P, 3).transpose(1, 0, 2).reshape(-1))
                # idx: per level wrap [16, ncolw] replicated x8,
                # stored partition-major [P, L, ncolw]
                ncolw = szw // 16
                iblk = np.empty((P, L, ncolw), np.int16)
                for l in range(L):
                    wrapped = loc[l].astype(np.int16).reshape(ncolw, 16).T
                    iblk[:, l, :] = np.tile(wrapped, (8, 1))
                idxs[c, io:io + P * L * ncolw] = iblk.reshape(-1)
                if n > 0:
                    vmask = np.arange(woff, woff + szw) < n
                    srcmap[c, pt0:pt0 + szw][vmask] = sel[:max(
                        0, min(n - woff, szw))]
                woff += szw
    return (xs, idxs, srcmap), counts


# -------------------------------------------------------------------- runner
def _make_runner(nc):
    import jax
    from jax.sharding import Mesh, PartitionSpec, NamedSharding
    from jax.experimental.shard_map import shard_map
    from concourse import bass2jax, mybir

    bass2jax.install_neuronx_cc_hook()
    partition_name = (nc.partition_id_tensor.name
                      if nc.partition_id_tensor else None)
    in_names, out_names, out_avals, out_shapes = [], [], [], []
    for alloc in nc.m.functions[0].allocations:
        if not isinstance(alloc, mybir.MemoryLocationSet):
            continue
        name = alloc.memorylocations[0].name
        if alloc.kind == "ExternalInput":
            if name != partition_name:
                in_names.append(name)
        elif alloc.kind == "ExternalOutput":
            shape = tuple(alloc.tensor_shape)
            dtype = mybir.dt.np(alloc.dtype)
            out_names.append(name)
            out_avals.append(jax.core.ShapedArray(shape, dtype))
            out_shapes.append((shape, dtype))
    n_params = len(in_names)
    all_in = list(in_names) + list(out_names)
    if partition_name is not None:
        all_in.append(partition_name)

    def _body(*args):
        operands = list(args)
        if partition_name is not None:
            operands.append(bass2jax.partition_id_tensor())
        outs = bass2jax._bass_exec_p.bind(
            *operands,
            out_avals=tuple(out_avals),
            in_names=tuple(all_in),
            out_names=tuple(out_names),
            lowering_input_output_aliases=(),
            sim_require_finite=True,
            sim_require_nnan=True,
            nc=nc,
        )
        return tuple(outs)

    devices = jax.devices()[:N_CORES]
    mesh = Mesh(np.asarray(devices), ("core",))
    spec = PartitionSpec("core")
    n_outs = len(out_names)
    sharded = jax.jit(
        shard_map(_body, mesh=mesh,
                  in_specs=(spec,) * (n_params + n_outs),
                  out_specs=(spec,) * n_outs, check_rep=False),
        keep_unused=True)
    shard = NamedSharding(mesh, spec)
    return sharded, shard, in_names, out_names, out_shapes


def _get_runner(caps):
    if caps not in _RUNNER_CACHE:
        _RUNNER_CACHE[caps] = _make_runner(_get_nc(caps))
    return _RUNNER_CACHE[caps]


# -------------------------------------------------------------------- kernel
def kernel(**inputs):
    import jax
    global _LAST_INFO

    x = np.ascontiguousarray(np.asarray(inputs["x"], dtype=np.float32))
    grids = [np.asarray(inputs[f"g{i}"], dtype=np.float32) for i in range(L)]
    assert x.shape == (N_POINTS, 3)

    caps = CAPS0
    wins, tot, x_elems, idx_elems, out_elems = _window_table(caps)
    prep, counts = _prep_core_inputs(x, caps, wins, tot, x_elems, idx_elems)
    if prep is None:
        # data-driven capacities (rare fallback; recompiles)
        caps = _capacities([math.ceil(c / N_CORES) * 1.05 + 64
                            for c in counts])
        wins, tot, x_elems, idx_elems, out_elems = _window_table(caps)
        prep, counts = _prep_core_inputs(x, caps, wins, tot, x_elems,
                                         idx_elems)
        assert prep is not None
    xs, idxs, srcmap = prep

    tabs = _make_tables(grids)
    sr = np.zeros((P, L), np.float32)
    for l, r in enumerate(RS):
        sr[:, l] = 0.5 * (r - 1)

    sharded, shard, in_names, out_names, out_shapes = _get_runner(caps)
    devices = jax.devices()[:N_CORES]

    def put_sharded(percore_list):
        g = tuple(
            (N_CORES * percore_list[0].shape[0],) + percore_list[0].shape[1:])
        parts = [jax.device_put(a, d) for a, d in zip(percore_list, devices)]
        return jax.make_array_from_single_device_arrays(g, shard, parts)

    per_core = {"x": xs, "idx": idxs}
    args = []
    for name in in_names:
        if name in per_core:
            args.append(put_sharded(
                [np.ascontiguousarray(per_core[name][c])
                 for c in range(N_CORES)]))
        elif name == "sr":
            args.append(put_sharded([sr] * N_CORES))
        elif name.startswith("t"):
            args.append(put_sharded([tabs[int(name[1])]] * N_CORES))
        else:
            raise KeyError(name)
    for shape, dtype in out_shapes:
        z = np.zeros(shape, dtype)
        args.append(put_sharded([z] * N_CORES))
    jax.block_until_ready(args)

    _LAST_INFO = {"caps": caps, "args": args, "sharded": sharded,
                  "out_shapes": out_shapes}

    outs = sharded(*args)
    jax.block_until_ready(outs)

    out_flat = np.asarray(outs[0]).reshape(N_CORES, out_elems)
    result = np.empty((N_POINTS, OC), np.float32)
    # per core: walk windows, de-permute p-major [P, Jw, OC] -> stream order
    for ci in range(N_CORES):
        oc = out_flat[ci]
        sm = srcmap[ci]
        for (szw, s2, s3, pt0, xo, io, oo, ncol) in wins:
            Jw = szw // P
            blk = oc[oo:oo + szw * OC].reshape(P, Jw, OC)
            ids = sm[pt0:pt0 + szw]
            valid = ids >= 0
            if valid.any():
                stream = blk.transpose(1, 0, 2).reshape(szw, OC)
                result[ids[valid]] = stream[valid]
    return np.ascontiguousarray(result)

